# revision 13
# baseline (speedup 1.0000x reference)
import os

os.environ.setdefault("MYCRO_LOCAL_CACHE", "1")

from contextlib import ExitStack

import numpy as np

import concourse.bass as bass
import concourse.tile as tile
from concourse import bacc, mybir
from concourse.masks import make_identity

C = 8
DCAP = 24
LAYERS = [(64, 64), (64, 128), (128, 256)]
SKIPS = [True, False, False]
HEADS = [("hd0", 256, 128), ("hd1", 128, 64), ("hd2", 64, 40)]

F32 = mybir.dt.float32
F32R = mybir.dt.float32r
I32 = mybir.dt.int32
AF = mybir.ActivationFunctionType
ALU = mybir.AluOpType

LAST_EXEC_TIME_NS = None
LAST_RESULTS = None
LAST_CFG = None
LAST_PERM = None


def _make_cfg(n_nodes, n_edges):
    npc = -(-(-(-n_nodes // C) // 128)) * 128  # ceil(ceil(n/C)/128)*128
    npc = ((n_nodes + C - 1) // C + 127) // 128 * 128
    chunks = []
    c0 = 0
    b0 = 0
    while c0 < npc:
        cn = min(512, npc - c0)
        nb = cn // 128
        chunks.append((c0, cn, b0, nb))
        c0 += cn
        b0 += nb
    return dict(N=n_nodes, E=n_edges, NPC=npc, NTOT=npc * C, NB=npc // 128,
                CHUNKS=chunks)


def _preprocess(x, src, dst, cfg):
    N = cfg["N"]
    E = cfg["E"]
    NPC = cfg["NPC"]
    NTOT = cfg["NTOT"]
    NB = cfg["NB"]
    deg = np.bincount(dst, minlength=N)
    order = np.argsort(-deg, kind="stable")
    rank = np.empty(N, np.int64)
    rank[order] = np.arange(N)
    new_of_old = (rank % C) * NPC + rank // C
    deg_rank = deg[order]
    Dbs = [max(int(deg_rank[min(b * C * 128, N - 1)]), 1) for b in range(NB)]

    src_new = new_of_old[src]
    dst_new = new_of_old[dst]
    eo = np.argsort(dst_new, kind="stable")
    ssrc = src_new[eo].astype(np.int32)
    counts = np.bincount(dst_new, minlength=NTOT).astype(np.int64)
    offs = np.zeros(NTOT + 1, np.int64)
    offs[1:] = np.cumsum(counts)

    idx_offs = []
    sumDs = []
    off = 0
    for (c0, cn, b0, nb) in cfg["CHUNKS"]:
        sumD = sum(Dbs[b0 + j] for j in range(nb))
        idx_offs.append(off)
        sumDs.append(sumD)
        off += 128 * sumD
    cfg["Dbs"] = Dbs
    cfg["idx_offs"] = idx_offs
    cfg["sumDs"] = sumDs
    cfg["SLOT"] = off

    idx_cores = []
    for c in range(C):
        parts = []
        for (c0, cn, b0, nb) in cfg["CHUNKS"]:
            bl = []
            for j in range(nb):
                ids = c * NPC + c0 + j * 128 + np.arange(128)
                d = counts[ids]
                D = Dbs[b0 + j]
                jj = np.arange(D)[None, :] % np.maximum(d, 1)[:, None]
                gi = np.minimum(offs[ids][:, None] + jj, E - 1)
                vals = ssrc[gi]
                vals = np.where((d == 0)[:, None], ids[:, None].astype(np.int32),
                                vals)
                bl.append(vals.astype(np.int32))
            parts.append(np.ascontiguousarray(
                np.concatenate(bl, axis=1)).reshape(-1))
        idx_cores.append(np.concatenate(parts))

    x_new = np.zeros((NTOT, x.shape[1]), np.float32)
    x_new[new_of_old] = x
    x_cores = [np.ascontiguousarray(x_new[c * NPC:(c + 1) * NPC].T)
               for c in range(C)]
    return x_cores, idx_cores, new_of_old


def _np(a):
    return np.asarray(a, np.float32)


def _lin_list(params):
    L = []
    L.append(("emb", _np(params["emb"]["w"]), _np(params["emb"]["b"])))
    for i, lay in enumerate(params["layers"]):
        F = LAYERS[i][1]
        for t, p in zip("abc", lay["mlp1"]):
            L.append((f"l{i}m1{t}", _np(p["w"]), _np(p["b"])))
        L.append((f"l{i}fc1", _np(lay["fc1"]["w"]), _np(lay["fc1"]["b"])))
        for t, p in zip("abc", lay["mlp2"]):
            L.append((f"l{i}m2{t}", _np(p["w"]), _np(p["b"])))
        w2 = _np(lay["fc2"]["w"])
        L.append((f"l{i}fc2fi", w2[:F], None))
        L.append((f"l{i}fc2a", w2[F:], _np(lay["fc2"]["b"])))
        for t, p in zip("abc", lay["mlp3"]):
            L.append((f"l{i}m3{t}", _np(p["w"]), _np(p["b"])))
    for (name, _, _), p in zip(HEADS, params["head"]):
        L.append((name, _np(p["w"]), _np(p["b"])))
    return L


def _pack(lins):
    wcols = []
    bcols = []
    wmeta = {}
    bmeta = {}
    wo = 0
    bo = 0
    for name, w, b in lins:
        fin, fout = w.shape
        if fin <= 128:
            ks = 1
            pin = fin
            pw = np.zeros((128, fout), np.float32)
            pw[:fin] = w
        else:
            assert fin % 128 == 0
            ks = fin // 128
            pin = 128
            pw = w.reshape(ks, 128, fout).transpose(1, 0, 2).reshape(
                128, ks * fout)
        wmeta[name] = (wo, ks, pin, fout)
        wcols.append(pw)
        wo += pw.shape[1]
        if b is not None:
            ms = (fout + 127) // 128
            pb = np.zeros((ms, 128), np.float32)
            pb.flat[:fout] = b
            bmeta[name] = (bo, ms)
            bcols.append(np.ascontiguousarray(pb.T))
            bo += ms
    WP = np.ascontiguousarray(np.concatenate(wcols, axis=1))
    BP = np.ascontiguousarray(np.concatenate(bcols, axis=1))
    return WP, BP, wmeta, bmeta


def _build(cfg, wmeta, bmeta, wtot, btot, finalize=True):
    NPC = cfg["NPC"]
    NTOT = cfg["NTOT"]
    dbg = bool(os.environ.get("DBG_DUMP"))
    nc = bacc.Bacc("TRN2", target_bir_lowering=False, debug=False,
                   enable_asserts=False, num_devices=C)
    x_t = nc.dram_tensor("x_fm", [6, NPC], F32, kind="ExternalInput")
    idx_t = nc.dram_tensor("idx_flat", [cfg["SLOT"]], I32,
                           kind="ExternalInput")
    wp_t = nc.dram_tensor("wpack", [128, wtot], F32, kind="ExternalInput")
    bp_t = nc.dram_tensor("bpack", [128, btot], F32, kind="ExternalInput")
    out_t = nc.dram_tensor("out_fm", [40, NPC], F32, kind="ExternalOutput")

    with tile.TileContext(nc) as tc:
        with ExitStack() as ctx:
            sb = ctx.enter_context(tc.tile_pool(name="sb", bufs=1))
            wkp = ctx.enter_context(tc.tile_pool(name="wkp", bufs=1))
            gpl = ctx.enter_context(tc.tile_pool(name="gpl", bufs=2))
            psp = ctx.enter_context(
                tc.tile_pool(name="psp", bufs=1, space="PSUM"))
            drp = ctx.enter_context(
                tc.tile_pool(name="drp", bufs=1, space="DRAM"))

            wsb = sb.tile([128, wtot], F32)
            bsb = sb.tile([128, btot], F32)
            ident = sb.tile([128, 128], F32)
            nc.sync.dma_start(wsb[:, :], wp_t[:, :])
            nc.sync.dma_start(bsb[:, :], bp_t[:, :])
            make_identity(nc, ident[:, :])

            def linear(wspecs, bname, rhs_list, cn, act, outv):
                fout = wmeta[wspecs[0]][3]
                ktot = sum(wmeta[n][1] for n in wspecs)
                mt = (fout + 127) // 128
                bo = bmeta[bname][0]
                for m in range(mt):
                    fm = min(128, fout - m * 128)
                    pt = psp.tile([128, 512], F32, tag="pmm", bufs=2)
                    ci = 0
                    for n in wspecs:
                        wo, ks, pin, fo = wmeta[n]
                        for k in range(ks):
                            co = wo + k * fo + m * 128
                            nc.tensor.matmul(
                                pt[:fm, :cn],
                                wsb[:pin, co:co + fm],
                                rhs_list[ci],
                                start=(ci == 0), stop=(ci == ktot - 1))
                            ci += 1
                    nc.scalar.activation(outv(m, fm), pt[:fm, :cn], act,
                                         bias=bsb[:fm, bo + m:bo + m + 1])

            nseq = [0]

            def vtile(F_, cn, tag, b=1):
                p = min(F_, 128)
                ks = max(F_ // 128, 1)
                nseq[0] += 1
                return wkp.tile([p, ks * cn], F32, tag=tag, bufs=b,
                                name=f"{tag}_{nseq[0]}")

            def vout(t, cn):
                return lambda m, fm: t[:fm, m * cn:m * cn + cn]

            def vch(t, F_, cn):
                p = min(F_, 128)
                ks = max(F_ // 128, 1)
                return [t[:p, k * cn:k * cn + cn] for k in range(ks)]

            def fm_load(t, dram, F_, c0, cn):
                p = min(F_, 128)
                ks = max(F_ // 128, 1)
                for k in range(ks):
                    nc.sync.dma_start(
                        t[:p, k * cn:k * cn + cn],
                        dram[:p, k * NPC + c0:k * NPC + c0 + cn])

            def fm_store(t, dram, F_, c0, cn):
                p = min(F_, 128)
                ks = max(F_ // 128, 1)
                for k in range(ks):
                    nc.gpsimd.dma_start(
                        dram[:p, k * NPC + c0:k * NPC + c0 + cn],
                        t[:p, k * cn:k * cn + cn])

            hd_prev = None
            dbg_copies = []
            for li, (di, F) in enumerate(LAYERS):
                pF = min(F, 128)
                ksF = max(F // 128, 1)
                f1_d = drp.tile([pF, ksF * NPC], F32)
                fifm_d = drp.tile([pF, ksF * NPC], F32)
                fish_d = drp.tile([NPC, F], F32)
                fifull_d = drp.tile([NTOT, F], F32)
                hd_next = drp.tile([pF, ksF * NPC], F32)
                if dbg:
                    for nm, src_t, shp in [
                            (f"dbg_f1_{li}", f1_d, [pF, ksF * NPC]),
                            (f"dbg_fi_{li}", fifm_d, [pF, ksF * NPC]),
                            (f"dbg_ff_{li}", fifull_d, [NTOT, F]),
                            (f"dbg_h_{li}", hd_next, [pF, ksF * NPC])]:
                        et = nc.dram_tensor(nm, shp, F32,
                                            kind="ExternalOutput")
                        dbg_copies.append((et, src_t))
                    dbg_mn = nc.dram_tensor(f"dbg_mn_{li}", [NPC, F], F32,
                                            kind="ExternalOutput")
                    dbg_agg = nc.dram_tensor(f"dbg_agg_{li}",
                                             [pF, ksF * NPC], F32,
                                             kind="ExternalOutput")
                    dbg_av = nc.dram_tensor(f"dbg_av_{li}",
                                            [pF, ksF * NPC], F32,
                                            kind="ExternalOutput")

                # Phase A: f1 = mlp1(h) (+h), fi = relu(fc1(f1)); store fi
                # node-major for the AllGather + gather table.
                for (c0, cn, b0, nb) in cfg["CHUNKS"]:
                    if li == 0:
                        xt = wkp.tile([6, 512], F32, tag="xin", bufs=2)
                        nc.sync.dma_start(xt[:6, :cn], x_t[:, c0:c0 + cn])
                        ht = vtile(di, cn, "hin", 2)
                        linear(["emb"], "emb", [xt[:6, :cn]], cn, AF.Identity,
                               vout(ht, cn))
                    else:
                        ht = vtile(di, cn, "hin", 2)
                        fm_load(ht, hd_prev, di, c0, cn)
                    t1 = vtile(64, cn, "t1")
                    linear([f"l{li}m1a"], f"l{li}m1a", vch(ht, di, cn), cn,
                           AF.Relu, vout(t1, cn))
                    t2 = vtile(128, cn, "t2")
                    linear([f"l{li}m1b"], f"l{li}m1b", vch(t1, 64, cn), cn,
                           AF.Relu, vout(t2, cn))
                    f1 = vtile(F, cn, "f1", 2)
                    linear([f"l{li}m1c"], f"l{li}m1c", vch(t2, 128, cn), cn,
                           AF.Identity, vout(f1, cn))
                    if SKIPS[li]:
                        nc.vector.tensor_add(f1[:pF, :ksF * cn],
                                             f1[:pF, :ksF * cn],
                                             ht[:pF, :ksF * cn])
                    fm_store(f1, f1_d, F, c0, cn)
                    fi = vtile(F, cn, "fi", 2)
                    linear([f"l{li}fc1"], f"l{li}fc1", vch(f1, F, cn), cn,
                           AF.Relu, vout(fi, cn))
                    fm_store(fi, fifm_d, F, c0, cn)
                    stage = wkp.tile([128, nb * F], F32, tag="stg", bufs=2)
                    for j in range(nb):
                        for k in range(ksF):
                            kf = min(128, F - k * 128)
                            pT = psp.tile([128, 128], F32, tag="pT", bufs=2)
                            nc.tensor.transpose(
                                pT[:128, :kf],
                                fi[:kf, k * cn + j * 128:k * cn + j * 128 + 128],
                                ident[:kf, :kf])
                            nc.scalar.copy(
                                stage[:, j * F + k * 128:j * F + k * 128 + kf],
                                pT[:128, :kf])
                    nc.gpsimd.dma_start(
                        fish_d[c0:c0 + cn, :].rearrange("(j p) f -> p j f",
                                                        p=128),
                        stage[:, :nb * F].rearrange("p (j f) -> p j f", f=F))

                nc.gpsimd.collective_compute(
                    "AllGather", ALU.bypass,
                    replica_groups=[list(range(C))],
                    ins=[fish_d.opt()],
                    outs=[fifull_d.opt()])

                # Phase C: agg = fi - segment_min(fi[src]); rest of the block.
                for ci, (c0, cn, b0, nb) in enumerate(cfg["CHUNKS"]):
                    sumD = cfg["sumDs"][ci]
                    io = cfg["idx_offs"][ci]
                    it = wkp.tile([128, sumD], I32, tag="idx", bufs=2)
                    nc.sync.dma_start(
                        it[:, :sumD],
                        idx_t[io:io + 128 * sumD].rearrange("(p d) -> p d",
                                                            d=sumD))
                    fit = vtile(F, cn, "fic", 2)
                    fm_load(fit, fifm_d, F, c0, cn)
                    agg = vtile(F, cn, "agg", 2)
                    cb = 0
                    for j in range(nb):
                        D = cfg["Dbs"][b0 + j]
                        mnm = wkp.tile([128, F], F32, tag="mnm", bufs=2)
                        p0 = 0
                        while p0 < D:
                            dp = min(DCAP, D - p0)
                            g = gpl.tile([128, dp * F], F32, tag="g")
                            for d in range(dp):
                                nc.gpsimd.indirect_dma_start(
                                    out=g[:, d * F:(d + 1) * F],
                                    out_offset=None,
                                    in_=fifull_d[:, :],
                                    in_offset=bass.IndirectOffsetOnAxis(
                                        ap=it[:, cb + p0 + d:cb + p0 + d + 1],
                                        axis=0))
                            if p0 == 0:
                                tgt = mnm
                            else:
                                tgt = wkp.tile([128, F], F32, tag="mtmp",
                                               bufs=2)
                            nc.vector.tensor_reduce(
                                tgt[:, :F],
                                g[:, :dp * F].rearrange("p (d f) -> p f d",
                                                        f=F),
                                mybir.AxisListType.X, ALU.min)
                            if p0 > 0:
                                nc.vector.tensor_tensor(
                                    mnm[:, :F], mnm[:, :F], tgt[:, :F],
                                    ALU.min)
                            p0 += dp
                        if dbg:
                            nc.gpsimd.dma_start(
                                dbg_mn[c0 + j * 128:c0 + j * 128 + 128, :],
                                mnm[:, :F])
                        for k in range(ksF):
                            kf = min(128, F - k * 128)
                            pT = psp.tile([128, 128], F32, tag="pT", bufs=2)
                            nc.tensor.transpose(pT[:kf, :128],
                                                mnm[:, k * 128:k * 128 + kf],
                                                ident[:, :])
                            nc.vector.tensor_sub(
                                agg[:kf, k * cn + j * 128:k * cn + j * 128 + 128],
                                fit[:kf, k * cn + j * 128:k * cn + j * 128 + 128],
                                pT[:kf, :128])
                        cb += D
                    if dbg:
                        fm_store(agg, dbg_agg, F, c0, cn)
                    a1 = vtile(32, cn, "a1")
                    linear([f"l{li}m2a"], f"l{li}m2a", vch(agg, F, cn), cn,
                           AF.Tanh, vout(a1, cn))
                    a2 = vtile(64, cn, "a2")
                    linear([f"l{li}m2b"], f"l{li}m2b", vch(a1, 32, cn), cn,
                           AF.Tanh, vout(a2, cn))
                    av = vtile(F, cn, "av")
                    linear([f"l{li}m2c"], f"l{li}m2c", vch(a2, 64, cn), cn,
                           AF.Identity, vout(av, cn))
                    if dbg:
                        fm_store(av, dbg_av, F, c0, cn)
                    h2 = vtile(F, cn, "h2")
                    linear([f"l{li}fc2fi", f"l{li}fc2a"], f"l{li}fc2a",
                           vch(fit, F, cn) + vch(av, F, cn), cn, AF.Relu,
                           vout(h2, cn))
                    f1t = vtile(F, cn, "f1c", 2)
                    fm_load(f1t, f1_d, F, c0, cn)
                    f2 = vtile(F, cn, "f2")
                    nc.vector.tensor_add(f2[:pF, :ksF * cn],
                                         h2[:pF, :ksF * cn],
                                         f1t[:pF, :ksF * cn])
                    m1 = vtile(128, cn, "m31")
                    linear([f"l{li}m3a"], f"l{li}m3a", vch(f2, F, cn), cn,
                           AF.Relu, vout(m1, cn))
                    m2t = vtile(64, cn, "m32")
                    linear([f"l{li}m3b"], f"l{li}m3b", vch(m1, 128, cn), cn,
                           AF.Relu, vout(m2t, cn))
                    m3s = vtile(F, cn, "m33")
                    linear([f"l{li}m3c"], f"l{li}m3c", vch(m2t, 64, cn), cn,
                           AF.Identity, vout(m3s, cn))
                    ho = vtile(F, cn, "ho", 2)
                    nc.vector.tensor_add(ho[:pF, :ksF * cn],
                                         m3s[:pF, :ksF * cn],
                                         f2[:pF, :ksF * cn])
                    fm_store(ho, hd_next, F, c0, cn)
                hd_prev = hd_next

            for (c0, cn, b0, nb) in cfg["CHUNKS"]:
                h3 = vtile(256, cn, "hin", 2)
                fm_load(h3, hd_prev, 256, c0, cn)
                d1 = vtile(128, cn, "d1")
                linear(["hd0"], "hd0", vch(h3, 256, cn), cn, AF.Tanh,
                       vout(d1, cn))
                d2 = vtile(64, cn, "d2")
                linear(["hd1"], "hd1", vch(d1, 128, cn), cn, AF.Relu,
                       vout(d2, cn))
                d3 = vtile(40, cn, "d3", 2)
                linear(["hd2"], "hd2", vch(d2, 64, cn), cn, AF.Identity,
                       vout(d3, cn))
                nc.gpsimd.dma_start(out_t[:, c0:c0 + cn], d3[:40, :cn])

            for et, src_t in dbg_copies:
                nc.sync.dma_start(et[:, :], src_t[:, :])

    if finalize:
        nc.compile()
    return nc


def _run_sim(nc, in_maps):
    from concourse import bass_interp
    nc.insert_bir_kernel_barrier_sem_inc()
    sim = bass_interp.MultiCoreSim(nc, C)
    for c in range(C):
        for k, v in in_maps[c].items():
            sim.cores[c].tensor(k)[:] = v
    sim.simulate()
    return [np.array(sim.cores[c].tensor("out_fm")) for c in range(C)]


def kernel(**inputs):
    global LAST_EXEC_TIME_NS
    x = np.asarray(inputs["x"], np.float32)
    ei = np.asarray(inputs["edge_index"])
    params = inputs["params"]
    cfg = _make_cfg(x.shape[0], ei.shape[1])
    x_cores, idx_cores, new_of_old = _preprocess(
        x, np.asarray(ei[0], np.int64), np.asarray(ei[1], np.int64), cfg)
    WP, BP, wmeta, bmeta = _pack(_lin_list(params))
    use_sim = bool(os.environ.get("BASS_SIM"))
    nc = _build(cfg, wmeta, bmeta, WP.shape[1], BP.shape[1],
                finalize=not use_sim)
    in_maps = [{"x_fm": x_cores[c], "idx_flat": idx_cores[c],
                "wpack": WP, "bpack": BP} for c in range(C)]
    if use_sim:
        outs = _run_sim(nc, in_maps)
    else:
        from concourse.bass_utils import run_bass_kernel_spmd
        res = run_bass_kernel_spmd(nc, in_maps, list(range(C)))
        LAST_EXEC_TIME_NS = res.exec_time_ns
        globals()["LAST_RESULTS"] = res.results
        globals()["LAST_CFG"] = cfg
        globals()["LAST_PERM"] = new_of_old
        outs = [res.results[c]["out_fm"] for c in range(C)]
    out_nm = np.concatenate([np.asarray(o, np.float32).T for o in outs],
                            axis=0)
    return np.ascontiguousarray(out_nm[new_of_old])
